# revision 73
# baseline (speedup 1.0000x reference)
"""Trainium2 Bass kernel: batched multi-head cross-attention.

Reference computation (per batch element b of 8, one NeuronCore each):
    K,V from x; Q from y (heads=16, dim=1024, d_head=64, scale=dim**-0.5)
    out = softmax(Q K^T * scale) V  -> concat heads -> @ w_out.T + b_out

Sharding: pure data-parallel on batch (8 batch elements -> 8 cores), no
collectives. All layout transposes / packings are host-side.

Design notes (what made it fast -- PE-stream-bound kernel, every matmul
is a uniform full-array background-weight-buffered MM at ~215ns/512
columns; avoid anything that breaks that stream):

  * Q/K projections run in fp8(e4m3) DoubleRow mode: 2 contraction
    planes per MM halves the projection slot count. Weights are
    pre-scaled by WS=64 into fp8's normal range; softmax scale and
    1/WS^2 are folded into the exp activation's free scale parameter.
    (x/y/wqk quantization adds ~1.6e-2 relative error -- inside the
    2e-2 budget; V / attnV / out-proj stay bf16.)
  * dots are plain K=128 MMs against per-head zero-padded K^T tiles
    (the other head's partition range is zero). Row-tiled K=64 pairing
    is ~27us cheaper on paper but pays ~100-190ns at every row-group
    switch (LDWEIGHTS can't be pulled ahead across a conflicting
    in-flight MM), which the ScalarE-paced psum cycling forces every
    couple of MMs -- full-array wins net.
  * attnV lhsT per head is [ones(64) | V_h(64)] (M=128): psum rows 0:64
    get the softmax denominator replicated across partitions, rows
    64:128 the unnormalized output. Normalize = reciprocal_approx_fast
    on psum[0:64] + one tensor_mul (psum[64:128] x rec -> OT), two
    same-engine DVE ops; the old s-hop/partition_broadcast chain's
    cross-engine waits used to head-of-line block the DVE FIFO.
  * PSUM: dots 2x[128,1024] (ScalarE-drain paced) / proj 1x[128,512] /
    attnV+vproj 3x[128,512] = exactly 8 banks.
  * Emission order = scheduler priority: per pair-period dots(t+1)
    first (keeps ScalarE fed), proj(t+2), then attnV(t)+normalize.
    wo loads late through spare ex-pool slots.
  * DMA: wqk8/x8/y8/wv are packed per-partition-contiguous so each is
    one large-descriptor transfer; x8+wv stream on the gpsimd queue in
    parallel with wqk+y8+xT on the sync queue (each dma_start costs
    ~650ns serially on its queue engine, so batch big and kick early).
"""

from contextlib import ExitStack

import numpy as np
import ml_dtypes

import concourse.bass as bass
import concourse.tile as tile
from concourse import bacc, mybir
from concourse.bass_utils import run_bass_kernel_spmd

DIM = 1024
N = 1024
HEADS = 16
DH = 64  # head dim
SCALE = DIM ** -0.5
P = 128          # partitions
NH = 512         # free-dim half (one PSUM bank of fp32)
BF16 = mybir.dt.bfloat16
F32 = mybir.dt.float32
F8 = mybir.dt.float8e4
DR = mybir.MatmulPerfMode.DoubleRow
EXP = mybir.ActivationFunctionType.Exp
WS = 64.0            # fp8 weight pre-scale (keeps wqk out of subnormals)
EXP_SCALE = SCALE / (WS * WS)


def build_attention_nc():
    nc = bacc.Bacc("TRN2", target_bir_lowering=False, debug=False)

    xT_d = nc.dram_tensor("xT", [DIM, N], BF16, kind="ExternalInput")
    # fp8 operands for the Q/K projections (DoubleRow: 2 contraction
    # planes per matmul). x8/y8: row a*128+p holds planes (c=2a, 2a+1)
    # interleaved along the free dim. wqk8: chunk (t, a) is a contiguous
    # [128, 512] block (plane-major [wq_t | wk_t] columns), scaled by WS
    # on the host (softmax scale + 1/WS^2 folded into the exp activation).
    x8_d = nc.dram_tensor("x8", [P, (DIM // 256) * 2 * N], F8,
                          kind="ExternalInput")
    y8_d = nc.dram_tensor("y8", [P, (DIM // 256) * 2 * N], F8,
                          kind="ExternalInput")
    wqk8_d = nc.dram_tensor("wqk8", [(DIM // P) * P, 4 * 512], F8,
                            kind="ExternalInput")
    wvT_d = nc.dram_tensor("wvT", [P, (DIM // P) * DIM], BF16,
                           kind="ExternalInput")
    woutT_d = nc.dram_tensor("woutT", [DIM, DIM], BF16, kind="ExternalInput")
    biasb_d = nc.dram_tensor("biasb", [P, DIM], F32, kind="ExternalInput")
    zpad_d = nc.dram_tensor("zpad", [DH, N], BF16, kind="ExternalInput")
    out_d = nc.dram_tensor("out", [N, DIM], F32, kind="ExternalOutput")

    CT = DIM // P   # 8 contraction tiles
    FT = DIM // P   # 8 feature tiles (per Q/K block) == head pairs
    JT = N // P     # 8 key-token tiles
    IT = N // P     # 8 query-token tiles

    with TileBuild(nc) as b:
        tc = b.tc
        ctx = b.ctx

        wqk_pool = ctx.enter_context(tc.tile_pool(name="wqk", bufs=4))
        xy_pool = ctx.enter_context(tc.tile_pool(name="xy", bufs=CT))
        qk_pool = ctx.enter_context(tc.tile_pool(name="qk", bufs=6))
        va_pool = ctx.enter_context(tc.tile_pool(name="va", bufs=JT))
        ex_pool = ctx.enter_context(tc.tile_pool(name="ex", bufs=36))
        ot_pool = ctx.enter_context(tc.tile_pool(name="ot", bufs=FT))
        sm_pool = ctx.enter_context(tc.tile_pool(name="sm", bufs=3))
        res_pool = ctx.enter_context(tc.tile_pool(name="res", bufs=2))
        pd_pool = ctx.enter_context(tc.tile_pool(name="pd", bufs=2, space="PSUM"))
        pp_pool = ctx.enter_context(tc.tile_pool(name="pp", bufs=1, space="PSUM"))
        pa_pool = ctx.enter_context(tc.tile_pool(name="pa", bufs=3, space="PSUM"))

        # ---- load inputs: wqk t=0 + y first so Q-proj / dots0 start
        # within ~3us, then x (K-proj + V-proj) + wv, then wqk t=1 ----
        wv_ctx = ExitStack()
        wv_pool = wv_ctx.enter_context(tc.tile_pool(name="wv", bufs=1))
        xT_t, yT_t = [], []
        wqk_t = {}

        CP = DIM // 256  # 4 contraction plane-pairs for fp8 DoubleRow

        def emit_wqk_chunks(t_blk):
            # one 256KB DMA per t-block, 2KB contiguous per partition
            t = wqk_pool.tile([P, CP, 2, 256], F8, tag="wqk",
                              name=f"wqk_{t_blk}")
            nc.sync.dma_start(
                t[:], wqk8_d[t_blk * P:(t_blk + 1) * P, :]
                .rearrange("p (a two f) -> p a two f", a=CP, two=2))
            wqk_t[t_blk] = t

        emit_wqk_chunks(0)
        # single-kick y8/x8: the early DMA window is descriptor-bound
        # (~128 descriptors per kick regardless of size), so one big
        # per-partition-contiguous transfer per tensor wins
        y8_t = xy_pool.tile([P, CP, 2, N], F8, tag="y8", name="y8", bufs=1)
        nc.sync.dma_start(
            y8_t[:], y8_d[:].rearrange("p (a two f) -> p a two f",
                                       a=CP, two=2))
        # wqk t=1..3 early (tiny transfers) so proj(1..3) can fill the
        # DMA-paced prologue with PE work
        for tb in (1, 2, 3):
            emit_wqk_chunks(tb)
        # x8 + wv on the gpsimd DMA queue, xT on sync: two parallel
        # streams, everything per-partition contiguous
        x8_t = xy_pool.tile([P, CP, 2, N], F8, tag="x8", name="x8", bufs=1)
        nc.gpsimd.dma_start(
            x8_t[:], x8_d[:].rearrange("p (a two f) -> p a two f",
                                       a=CP, two=2))
        wv_t = wv_pool.tile([P, CT, DIM], BF16, tag="wv", name="wv")
        nc.gpsimd.dma_start(
            wv_t[:], wvT_d[:].rearrange("p (c f) -> p c f", c=CT))
        for c in range(CT):
            t = xy_pool.tile([P, N], BF16, tag="xy", name=f"xt{c}")
            nc.sync.dma_start(t[:], xT_d[c * P:(c + 1) * P, :])
            xT_t.append(t)

        wo_t = []

        def emit_bias():
            bias_t = res_pool.tile([P, DIM], F32, tag="bias", bufs=1)
            nc.sync.dma_start(bias_t[:], biasb_d[:, :])
            return bias_t

        def emit_wo(fs):
            # wo tiles borrow ex-pool slots (same shape/dtype); loaded in
            # two half-batches so the 8-slot demand never lands at once
            # (a full batch starves ScalarE of ex slots for ~1.5us)
            for f in fs:
                t = ex_pool.tile([P, DIM], BF16, tag="ex", name=f"wo{f}")
                nc.sync.dma_start(t[:], woutT_d[f * P:(f + 1) * P, :])
                wo_t.append(t)

        # ---- V projection: c-major streaming groups (MMs begin once the
        # first x/wv c-tiles arrive), token-major + per-head ones col ----
        VA = []

        def emit_vproj():
            # half-bank groups on the pp pool (like proj): vproj is the
            # low-priority PE gap-filler and must not contend for the
            # dots psum slots in pd.
            HH = HEADS // 2
            for j in range(JT):
                # per head h: cols [ones(64) | V_h(64)] contiguous, so the
                # attnv lhsT is a single [128,128] slice that replicates
                # the softmax denominator across psum partitions 0:64
                va = va_pool.tile([P, HEADS, 2, DH], BF16, tag="va",
                                  name=f"va{j}")
                nc.vector.memset(va[:, :, 0, :], 1.0)
                for n in range(2):
                    ps = pa_pool.tile([P, NH], F32, tag="pa",
                                      name=f"psv{j}_{n}")
                    for c in range(CT):
                        nc.tensor.matmul(
                            ps[:],
                            lhsT=xT_t[c][:, j * P:(j + 1) * P],
                            rhs=wv_t[:, c, n * NH:(n + 1) * NH],
                            start=(c == 0), stop=(c == CT - 1),
                        )
                    nc.vector.tensor_copy(
                        va[:, n * HH:(n + 1) * HH, 1, :],
                        ps[:].rearrange("p (h c) -> p h c", c=DH),
                    )
                VA.append(va)

        # ---- Q/K projection for head pair t ----
        # Q: one [128,1024] tile (both heads' d-strips; used whole as the
        # dots rhs). K: per-head zero-padded [128,1024] tiles -- the other
        # head's partition range is zeroed so the dots matmul is a plain
        # full-array K=128 MM (background-buffered LDWEIGHTS, no row-group
        # switch penalties anywhere in the kernel).
        def emit_proj(t):
            # half-size (1 PSUM bank) groups on a 2-slot pool: the next
            # 8-MM group streams while the previous group's DVE cast is
            # still queued, so a busy DVE never stalls the PE here.
            # dead partition halves of the kt tiles are zero-filled by DMA
            # from a host zeros tensor (DMA engines are idle mid-phase;
            # keeps the zeroing off the busy DVE)
            qt = qk_pool.tile([P, N], BF16, tag="qk", name=f"qkq_{t}",
                              bufs=2)
            kt = [qk_pool.tile([P, N], BF16, tag=f"kt{par}",
                               name=f"qkk{par}_{t}", bufs=2)
                  for par in range(2)]
            nc.sync.dma_start(kt[0][DH:P, :], zpad_d[:, :])
            nc.sync.dma_start(kt[1][0:DH, :], zpad_d[:, :])
            for which, rhs_t in ((0, y8_t), (1, x8_t)):
                for n in range(2):
                    ps = pp_pool.tile([P, NH], F32, tag="pp",
                                      name=f"psp{which}_{t}_{n}")
                    for a in range(CP):
                        nc.tensor.matmul(
                            ps[:],
                            lhsT=wqk_t[t][:, a, :,
                                          which * P:(which + 1) * P],
                            rhs=rhs_t[:, a, :, n * NH:(n + 1) * NH],
                            start=(a == 0), stop=(a == CP - 1),
                            perf_mode=DR,
                        )
                    if which == 0:
                        nc.vector.tensor_copy(qt[:, n * NH:(n + 1) * NH],
                                              ps[:])
                    else:
                        nc.vector.tensor_copy(
                            kt[0][0:DH, n * NH:(n + 1) * NH], ps[0:DH, :])
                        nc.vector.tensor_copy(
                            kt[1][DH:P, n * NH:(n + 1) * NH], ps[DH:P, :])
            return qt, kt

        # ---- dots + exp for head pair t: plain full-array K=128 MMs
        # against the zero-padded per-head K tiles; one [128,1024] psum +
        # one exp activation per (head, j) ----
        def emit_dots_exp_pair(t, QTt, KTt):
            ex_l = {0: [], 1: []}
            for j in range(JT):
                for par in range(2):
                    ps = pd_pool.tile([P, N], F32, tag="pd",
                                      name=f"psd{t}_{j}_{par}")
                    for n in range(2):
                        nc.tensor.matmul(
                            ps[:, n * NH:(n + 1) * NH],
                            lhsT=KTt[par][:, j * P:(j + 1) * P],
                            rhs=QTt[:, n * NH:(n + 1) * NH],
                            start=True, stop=True,
                        )
                    ex = ex_pool.tile([P, N], BF16, tag="ex",
                                      name=f"ex{t}_{j}_{par}")
                    nc.scalar.activation(ex[:], ps[:], EXP, scale=EXP_SCALE)
                    ex_l[par].append(ex)
            return ex_l

        OT = [ot_pool.tile([P, N], BF16, tag="ot", name=f"OT{f}")
              for f in range(FT)]

        # ---- attn@V for one head: lhsT = [ones x64 | V_h] so psum rows
        # 0:64 hold the softmax denominator replicated across partitions
        # and rows 64:128 hold the unnormalized output ----
        def emit_attnv(t, par, ex_l):
            h = 2 * t + par
            accs = [pa_pool.tile([P, NH], F32, tag="pa", name=f"acc{h}_{n}")
                    for n in range(2)]
            for j in range(JT):
                for n in range(2):
                    nc.tensor.matmul(
                        accs[n][:],
                        lhsT=VA[j][:, h, :, :],
                        rhs=ex_l[j][:, n * NH:(n + 1) * NH],
                        start=(j == 0), stop=(j == JT - 1),
                    )
            return accs

        # ---- softmax normalize: reciprocal of the replicated denominator
        # rows then one multiply that also evacuates the psum -- two
        # same-engine DVE ops, no cross-engine ping-pong ----
        def emit_norm(t, par, accs):
            h = 2 * t + par
            pb = par * DH
            for n in range(2):
                rec = sm_pool.tile([DH, NH], F32, tag="rec",
                                   name=f"rec{h}_{n}", bufs=2)
                nc.vector.reciprocal_approx_fast(
                    out=rec[:], in_=accs[n][0:DH, :])
                nc.vector.tensor_mul(
                    OT[t][pb:pb + DH, n * NH:(n + 1) * NH],
                    accs[n][DH:P, :], rec[:])

        # ---- emission schedule (= scheduler priority order): get the
        # ScalarE exp stream started ASAP (proj0 -> proj1 -> dots0), with
        # V-proj demoted to PE gap-filler; then per pair: dots one PAIR
        # ahead FIRST (keeps ScalarE fed), proj two ahead as filler,
        # attnV last (its results are only needed at the end) ----
        proj_bufs = {0: emit_proj(0), 1: emit_proj(1)}
        ex_pair = emit_dots_exp_pair(0, *proj_bufs[0])
        emit_vproj()
        wv_ctx.close()
        bias_t = emit_bias()
        for t in range(FT):
            nxt = (emit_dots_exp_pair(t + 1, *proj_bufs[t + 1])
                   if t + 1 < FT else None)
            if t + 2 < FT:
                if t + 2 >= 4:
                    emit_wqk_chunks(t + 2)
                proj_bufs[t + 2] = emit_proj(t + 2)
            if t in (2, 3, 5, 6):
                k = {2: 0, 3: 1, 5: 2, 6: 3}[t]
                emit_wo(range(k * 2, k * 2 + 2))
            a0 = emit_attnv(t, 0, ex_pair[0])
            emit_norm(t, 0, a0)
            a1 = emit_attnv(t, 1, ex_pair[1])
            emit_norm(t, 1, a1)
            ex_pair = nxt

        # ---- output projection + bias ----
        for i in range(IT):
            ps = pd_pool.tile([P, N], F32, tag="pd", name=f"psf{i}")
            for f in range(FT):
                for n in range(2):
                    nc.tensor.matmul(
                        ps[:, n * NH:(n + 1) * NH],
                        lhsT=OT[f][:, i * P:(i + 1) * P],
                        rhs=wo_t[f][:, n * NH:(n + 1) * NH],
                        start=(f == 0), stop=(f == FT - 1),
                    )
            # the end barrier waits for the LAST out-store to complete at
            # slow end-window DMA rate (~75GB/s): the final two i-tiles
            # store in quarter-chunks, with the last tile on the idle
            # gpsimd queue so the two trailing transfers run in parallel
            res = res_pool.tile([P, DIM], F32, tag="res", name=f"res{i}")
            nchunk = 4 if i >= IT - 2 else 2
            q = nc.gpsimd if i == IT - 1 else nc.sync
            CW = DIM // nchunk
            for hf in range(nchunk):
                sl = slice(hf * CW, (hf + 1) * CW)
                nc.vector.tensor_add(res[:, sl], ps[:, sl], bias_t[:, sl])
                q.dma_start(out=out_d[i * P:(i + 1) * P, sl],
                            in_=res[:, sl])

    nc.compile()
    return nc


class TileBuild:
    """TileContext + ExitStack pools in one with-block."""

    def __init__(self, nc):
        self.nc = nc
        self.ctx = ExitStack()
        self._tc_cm = tile.TileContext(nc)

    def __enter__(self):
        self.tc = self._tc_cm.__enter__()
        self.ctx.__enter__()
        return self

    def __exit__(self, *exc):
        self.ctx.__exit__(*exc)
        return self._tc_cm.__exit__(*exc)


_NC_CACHE = None


def _get_nc():
    global _NC_CACHE
    if _NC_CACHE is None:
        _NC_CACHE = build_attention_nc()
    return _NC_CACHE


def prepare_inputs(x, y, w_qkv, w_out, b_out):
    bf16 = ml_dtypes.bfloat16
    xT32 = np.ascontiguousarray(np.transpose(x, (0, 2, 1))).astype(np.float32)
    yT32 = np.ascontiguousarray(np.transpose(y, (0, 2, 1))).astype(np.float32)
    xT = xT32.astype(bf16)
    f8 = ml_dtypes.float8_e4m3
    wqkvT = np.ascontiguousarray(np.array(w_qkv, dtype=np.float32).T)
    # fp8 DoubleRow packing for the Q/K projection weights, scaled by WS
    # (softmax scale and 1/WS^2 are folded into the exp activation):
    # wqk8[t, a, p, plane, m] = WS * wqkvT[256a + 128*plane + p, col_t[m]]
    w8 = np.empty((DIM // P, DIM // 256, P, 2, 256), dtype=np.float32)
    for t in range(DIM // P):
        blk = np.concatenate(
            [wqkvT[:, t * P:(t + 1) * P],
             wqkvT[:, DIM + t * P:DIM + (t + 1) * P]], axis=1)  # [dim, 256]
        w8[t] = (WS * blk).reshape(DIM // 256, 2, P, 256).transpose(0, 2, 1, 3)
    # per-partition contiguous: row (t, p) = [a, plane, m]
    wqk8 = np.ascontiguousarray(
        w8.transpose(0, 2, 1, 3, 4).reshape((DIM // P) * P, 4 * 512)).astype(f8)
    wvT = np.ascontiguousarray(
        wqkvT[:, 2 * DIM:3 * DIM].reshape(DIM // P, P, DIM)
        .transpose(1, 0, 2).reshape(P, (DIM // P) * DIM)).astype(bf16)
    woutT = np.ascontiguousarray(np.array(w_out, dtype=np.float32).T).astype(bf16)
    biasb = np.ascontiguousarray(
        np.broadcast_to(np.array(b_out, dtype=np.float32), (P, DIM)))
    in_maps = []
    for i in range(x.shape[0]):
        in_maps.append({
            "xT": np.ascontiguousarray(xT[i]),
            "x8": np.ascontiguousarray(
                xT32[i].reshape(DIM // 256, 2, P, N)
                .transpose(2, 0, 1, 3).reshape(P, (DIM // 256) * 2 * N)
                ).astype(f8),
            "y8": np.ascontiguousarray(
                yT32[i].reshape(DIM // 256, 2, P, N)
                .transpose(2, 0, 1, 3).reshape(P, (DIM // 256) * 2 * N)
                ).astype(f8),
            "wqk8": wqk8,
            "wvT": wvT,
            "woutT": woutT,
            "biasb": biasb,
            "zpad": np.zeros((DH, N), dtype=bf16),
        })
    return in_maps


def kernel(x, y, w_qkv, w_out, b_out, trace=False):
    nc = _get_nc()
    in_maps = prepare_inputs(x, y, w_qkv, w_out, b_out)
    r = run_bass_kernel_spmd(nc, in_maps, core_ids=list(range(len(in_maps))),
                             trace=trace)
    out = np.stack([r.results[i]["out"] for i in range(len(in_maps))])
    if trace:
        kernel.last_results = r
    return out.astype(np.float32)



# revision 76
# speedup vs baseline: 1.0100x; 1.0100x over previous
"""Trainium2 Bass kernel: batched multi-head cross-attention.

Reference computation (per batch element b of 8, one NeuronCore each):
    K,V from x; Q from y (heads=16, dim=1024, d_head=64, scale=dim**-0.5)
    out = softmax(Q K^T * scale) V  -> concat heads -> @ w_out.T + b_out

Sharding: pure data-parallel on batch (8 batch elements -> 8 cores), no
collectives. All layout transposes / packings are host-side.

Design notes (what made it fast -- PE-stream-bound kernel, every matmul
is a uniform full-array background-weight-buffered MM at ~215ns/512
columns; avoid anything that breaks that stream):

  * Q/K projections run in fp8(e4m3) DoubleRow mode: 2 contraction
    planes per MM halves the projection slot count. Weights are
    pre-scaled by WS=64 into fp8's normal range; softmax scale and
    1/WS^2 are folded into the exp activation's free scale parameter.
    (x/y/wqk quantization adds ~1.6e-2 relative error -- inside the
    2e-2 budget; V / attnV / out-proj stay bf16.)
  * dots are plain K=128 MMs against per-head zero-padded K^T tiles
    (the other head's partition range is zero). Row-tiled K=64 pairing
    is ~27us cheaper on paper but pays ~100-190ns at every row-group
    switch (LDWEIGHTS can't be pulled ahead across a conflicting
    in-flight MM), which the ScalarE-paced psum cycling forces every
    couple of MMs -- full-array wins net.
  * attnV lhsT per head is [ones(64) | V_h(64)] (M=128): psum rows 0:64
    get the softmax denominator replicated across partitions, rows
    64:128 the unnormalized output. Normalize = reciprocal_approx_fast
    on psum[0:64] + one tensor_mul (psum[64:128] x rec -> OT), two
    same-engine DVE ops; the old s-hop/partition_broadcast chain's
    cross-engine waits used to head-of-line block the DVE FIFO.
  * PSUM: dots 2x[128,1024] (ScalarE-drain paced) / proj 1x[128,512] /
    attnV+vproj 3x[128,512] = exactly 8 banks.
  * Emission order = scheduler priority: per pair-period dots(t+1)
    first (keeps ScalarE fed), proj(t+2), then attnV(t)+normalize.
    wo loads late through spare ex-pool slots.
  * DMA: wqk8/x8/y8/wv are packed per-partition-contiguous so each is
    one large-descriptor transfer; x8+wv stream on the gpsimd queue in
    parallel with wqk+y8+xT on the sync queue (each dma_start costs
    ~650ns serially on its queue engine, so batch big and kick early).
"""

from contextlib import ExitStack

import numpy as np
import ml_dtypes

import concourse.bass as bass
import concourse.tile as tile
from concourse import bacc, mybir
from concourse.bass_utils import run_bass_kernel_spmd

DIM = 1024
N = 1024
HEADS = 16
DH = 64  # head dim
SCALE = DIM ** -0.5
P = 128          # partitions
NH = 512         # free-dim half (one PSUM bank of fp32)
BF16 = mybir.dt.bfloat16
F32 = mybir.dt.float32
F8 = mybir.dt.float8e4
DR = mybir.MatmulPerfMode.DoubleRow
EXP = mybir.ActivationFunctionType.Exp
WS = 64.0            # fp8 weight pre-scale (keeps wqk out of subnormals)
EXP_SCALE = SCALE / (WS * WS)


def build_attention_nc():
    nc = bacc.Bacc("TRN2", target_bir_lowering=False, debug=False)

    xT_d = nc.dram_tensor("xT", [DIM, N], BF16, kind="ExternalInput")
    # fp8 operands for the Q/K projections (DoubleRow: 2 contraction
    # planes per matmul). x8/y8: row a*128+p holds planes (c=2a, 2a+1)
    # interleaved along the free dim. wqk8: chunk (t, a) is a contiguous
    # [128, 512] block (plane-major [wq_t | wk_t] columns), scaled by WS
    # on the host (softmax scale + 1/WS^2 folded into the exp activation).
    x8_d = nc.dram_tensor("x8", [P, (DIM // 256) * 2 * N], F8,
                          kind="ExternalInput")
    y8_d = nc.dram_tensor("y8", [P, (DIM // 256) * 2 * N], F8,
                          kind="ExternalInput")
    wqk8_d = nc.dram_tensor("wqk8", [(DIM // P) * P, 4 * 512], F8,
                            kind="ExternalInput")
    wvT_d = nc.dram_tensor("wvT", [P, (DIM // P) * DIM], BF16,
                           kind="ExternalInput")
    woutT_d = nc.dram_tensor("woutT", [DIM, DIM], BF16, kind="ExternalInput")
    biasb_d = nc.dram_tensor("biasb", [P, DIM], F32, kind="ExternalInput")
    zpad_d = nc.dram_tensor("zpad", [DH, N], BF16, kind="ExternalInput")
    out_d = nc.dram_tensor("out", [N, DIM], F32, kind="ExternalOutput")

    CT = DIM // P   # 8 contraction tiles
    FT = DIM // P   # 8 feature tiles (per Q/K block) == head pairs
    JT = N // P     # 8 key-token tiles
    IT = N // P     # 8 query-token tiles

    with TileBuild(nc) as b:
        tc = b.tc
        ctx = b.ctx

        wqk_pool = ctx.enter_context(tc.tile_pool(name="wqk", bufs=4))
        xy_pool = ctx.enter_context(tc.tile_pool(name="xy", bufs=CT))
        qk_pool = ctx.enter_context(tc.tile_pool(name="qk", bufs=6))
        va_pool = ctx.enter_context(tc.tile_pool(name="va", bufs=JT))
        ex_pool = ctx.enter_context(tc.tile_pool(name="ex", bufs=37))
        ot_pool = ctx.enter_context(tc.tile_pool(name="ot", bufs=FT))
        sm_pool = ctx.enter_context(tc.tile_pool(name="sm", bufs=3))
        res_pool = ctx.enter_context(tc.tile_pool(name="res", bufs=2))
        pd_pool = ctx.enter_context(tc.tile_pool(name="pd", bufs=2, space="PSUM"))
        pp_pool = ctx.enter_context(tc.tile_pool(name="pp", bufs=1, space="PSUM"))
        pa_pool = ctx.enter_context(tc.tile_pool(name="pa", bufs=3, space="PSUM"))

        # ---- load inputs: wqk t=0 + y first so Q-proj / dots0 start
        # within ~3us, then x (K-proj + V-proj) + wv, then wqk t=1 ----
        wv_ctx = ExitStack()
        wv_pool = wv_ctx.enter_context(tc.tile_pool(name="wv", bufs=1))
        xT_t, yT_t = [], []
        wqk_t = {}

        CP = DIM // 256  # 4 contraction plane-pairs for fp8 DoubleRow

        def emit_wqk_chunks(t_blk):
            # one 256KB DMA per t-block, 2KB contiguous per partition
            t = wqk_pool.tile([P, CP, 2, 256], F8, tag="wqk",
                              name=f"wqk_{t_blk}")
            nc.sync.dma_start(
                t[:], wqk8_d[t_blk * P:(t_blk + 1) * P, :]
                .rearrange("p (a two f) -> p a two f", a=CP, two=2))
            wqk_t[t_blk] = t

        emit_wqk_chunks(0)
        # single-kick y8/x8: the early DMA window is descriptor-bound
        # (~128 descriptors per kick regardless of size), so one big
        # per-partition-contiguous transfer per tensor wins
        y8_t = xy_pool.tile([P, CP, 2, N], F8, tag="y8", name="y8", bufs=1)
        nc.sync.dma_start(
            y8_t[:], y8_d[:].rearrange("p (a two f) -> p a two f",
                                       a=CP, two=2))
        # wqk t=1..3 early (tiny transfers) so proj(1..3) can fill the
        # DMA-paced prologue with PE work
        for tb in (1, 2, 3):
            emit_wqk_chunks(tb)
        # x8 + wv on the gpsimd DMA queue, xT on sync: two parallel
        # streams, everything per-partition contiguous
        x8_t = xy_pool.tile([P, CP, 2, N], F8, tag="x8", name="x8", bufs=1)
        nc.gpsimd.dma_start(
            x8_t[:], x8_d[:].rearrange("p (a two f) -> p a two f",
                                       a=CP, two=2))
        wv_t = wv_pool.tile([P, CT, DIM], BF16, tag="wv", name="wv")
        nc.gpsimd.dma_start(
            wv_t[:], wvT_d[:].rearrange("p (c f) -> p c f", c=CT))
        for c in range(CT):
            t = xy_pool.tile([P, N], BF16, tag="xy", name=f"xt{c}")
            nc.sync.dma_start(t[:], xT_d[c * P:(c + 1) * P, :])
            xT_t.append(t)

        wo_t = []

        def emit_bias():
            bias_t = res_pool.tile([P, DIM], F32, tag="bias", bufs=1)
            nc.sync.dma_start(bias_t[:], biasb_d[:, :])
            return bias_t

        def emit_wo(fs):
            # wo tiles borrow ex-pool slots (same shape/dtype); loaded in
            # two half-batches so the 8-slot demand never lands at once
            # (a full batch starves ScalarE of ex slots for ~1.5us)
            for f in fs:
                t = ex_pool.tile([P, DIM], BF16, tag="ex", name=f"wo{f}")
                nc.sync.dma_start(t[:], woutT_d[f * P:(f + 1) * P, :])
                wo_t.append(t)

        # ---- V projection: c-major streaming groups (MMs begin once the
        # first x/wv c-tiles arrive), token-major + per-head ones col ----
        VA = []

        def emit_vproj():
            # half-bank groups on the pp pool (like proj): vproj is the
            # low-priority PE gap-filler and must not contend for the
            # dots psum slots in pd.
            HH = HEADS // 2
            for j in range(JT):
                # per head h: cols [ones(64) | V_h(64)] contiguous, so the
                # attnv lhsT is a single [128,128] slice that replicates
                # the softmax denominator across psum partitions 0:64
                va = va_pool.tile([P, HEADS, 2, DH], BF16, tag="va",
                                  name=f"va{j}")
                nc.vector.memset(va[:, :, 0, :], 1.0)
                for n in range(2):
                    ps = pa_pool.tile([P, NH], F32, tag="pa",
                                      name=f"psv{j}_{n}")
                    for c in range(CT):
                        nc.tensor.matmul(
                            ps[:],
                            lhsT=xT_t[c][:, j * P:(j + 1) * P],
                            rhs=wv_t[:, c, n * NH:(n + 1) * NH],
                            start=(c == 0), stop=(c == CT - 1),
                        )
                    nc.vector.tensor_copy(
                        va[:, n * HH:(n + 1) * HH, 1, :],
                        ps[:].rearrange("p (h c) -> p h c", c=DH),
                    )
                VA.append(va)

        # ---- Q/K projection for head pair t ----
        # Q: one [128,1024] tile (both heads' d-strips; used whole as the
        # dots rhs). K: per-head zero-padded [128,1024] tiles -- the other
        # head's partition range is zeroed so the dots matmul is a plain
        # full-array K=128 MM (background-buffered LDWEIGHTS, no row-group
        # switch penalties anywhere in the kernel).
        def emit_proj(t):
            # half-size (1 PSUM bank) groups on a 2-slot pool: the next
            # 8-MM group streams while the previous group's DVE cast is
            # still queued, so a busy DVE never stalls the PE here.
            # dead partition halves of the kt tiles are zero-filled by DMA
            # from a host zeros tensor (DMA engines are idle mid-phase;
            # keeps the zeroing off the busy DVE)
            qt = qk_pool.tile([P, N], BF16, tag="qk", name=f"qkq_{t}",
                              bufs=2)
            kt = [qk_pool.tile([P, N], BF16, tag=f"kt{par}",
                               name=f"qkk{par}_{t}", bufs=2)
                  for par in range(2)]
            nc.sync.dma_start(kt[0][DH:P, :], zpad_d[:, :])
            nc.sync.dma_start(kt[1][0:DH, :], zpad_d[:, :])
            for which, rhs_t in ((0, y8_t), (1, x8_t)):
                for n in range(2):
                    ps = pp_pool.tile([P, NH], F32, tag="pp",
                                      name=f"psp{which}_{t}_{n}")
                    for a in range(CP):
                        nc.tensor.matmul(
                            ps[:],
                            lhsT=wqk_t[t][:, a, :,
                                          which * P:(which + 1) * P],
                            rhs=rhs_t[:, a, :, n * NH:(n + 1) * NH],
                            start=(a == 0), stop=(a == CP - 1),
                            perf_mode=DR,
                        )
                    if which == 0:
                        nc.vector.tensor_copy(qt[:, n * NH:(n + 1) * NH],
                                              ps[:])
                    else:
                        nc.vector.tensor_copy(
                            kt[0][0:DH, n * NH:(n + 1) * NH], ps[0:DH, :])
                        nc.vector.tensor_copy(
                            kt[1][DH:P, n * NH:(n + 1) * NH], ps[DH:P, :])
            return qt, kt

        # ---- dots + exp for head pair t: plain full-array K=128 MMs
        # against the zero-padded per-head K tiles; one [128,1024] psum +
        # one exp activation per (head, j) ----
        def emit_dots_exp_pair(t, QTt, KTt):
            ex_l = {0: [], 1: []}
            for j in range(JT):
                for par in range(2):
                    ps = pd_pool.tile([P, N], F32, tag="pd",
                                      name=f"psd{t}_{j}_{par}")
                    for n in range(2):
                        nc.tensor.matmul(
                            ps[:, n * NH:(n + 1) * NH],
                            lhsT=KTt[par][:, j * P:(j + 1) * P],
                            rhs=QTt[:, n * NH:(n + 1) * NH],
                            start=True, stop=True,
                        )
                    ex = ex_pool.tile([P, N], BF16, tag="ex",
                                      name=f"ex{t}_{j}_{par}")
                    nc.scalar.activation(ex[:], ps[:], EXP, scale=EXP_SCALE)
                    ex_l[par].append(ex)
            return ex_l

        OT = [ot_pool.tile([P, N], BF16, tag="ot", name=f"OT{f}")
              for f in range(FT)]

        # ---- attn@V for one head: lhsT = [ones x64 | V_h] so psum rows
        # 0:64 hold the softmax denominator replicated across partitions
        # and rows 64:128 hold the unnormalized output ----
        def emit_attnv(t, par, ex_l):
            h = 2 * t + par
            accs = [pa_pool.tile([P, NH], F32, tag="pa", name=f"acc{h}_{n}")
                    for n in range(2)]
            for j in range(JT):
                for n in range(2):
                    nc.tensor.matmul(
                        accs[n][:],
                        lhsT=VA[j][:, h, :, :],
                        rhs=ex_l[j][:, n * NH:(n + 1) * NH],
                        start=(j == 0), stop=(j == JT - 1),
                    )
            return accs

        # ---- softmax normalize: reciprocal of the replicated denominator
        # rows then one multiply that also evacuates the psum -- two
        # same-engine DVE ops, no cross-engine ping-pong ----
        def emit_norm(t, par, accs):
            h = 2 * t + par
            pb = par * DH
            for n in range(2):
                rec = sm_pool.tile([DH, NH], F32, tag="rec",
                                   name=f"rec{h}_{n}", bufs=2)
                nc.vector.reciprocal_approx_fast(
                    out=rec[:], in_=accs[n][0:DH, :])
                nc.vector.tensor_mul(
                    OT[t][pb:pb + DH, n * NH:(n + 1) * NH],
                    accs[n][DH:P, :], rec[:])

        # ---- emission schedule (= scheduler priority order): get the
        # ScalarE exp stream started ASAP (proj0 -> proj1 -> dots0), with
        # V-proj demoted to PE gap-filler; then per pair: dots one PAIR
        # ahead FIRST (keeps ScalarE fed), proj two ahead as filler,
        # attnV last (its results are only needed at the end) ----
        proj_bufs = {0: emit_proj(0), 1: emit_proj(1)}
        ex_pair = emit_dots_exp_pair(0, *proj_bufs[0])
        emit_vproj()
        wv_ctx.close()
        bias_t = emit_bias()
        for t in range(FT):
            nxt = (emit_dots_exp_pair(t + 1, *proj_bufs[t + 1])
                   if t + 1 < FT else None)
            if t + 2 < FT:
                if t + 2 >= 4:
                    emit_wqk_chunks(t + 2)
                proj_bufs[t + 2] = emit_proj(t + 2)
            if t == 3:
                emit_wo(range(FT // 2))
            if t == 5:
                emit_wo(range(FT // 2, FT))
            a0 = emit_attnv(t, 0, ex_pair[0])
            emit_norm(t, 0, a0)
            a1 = emit_attnv(t, 1, ex_pair[1])
            emit_norm(t, 1, a1)
            ex_pair = nxt

        # ---- output projection + bias ----
        for i in range(IT):
            ps = pd_pool.tile([P, N], F32, tag="pd", name=f"psf{i}")
            for f in range(FT):
                for n in range(2):
                    nc.tensor.matmul(
                        ps[:, n * NH:(n + 1) * NH],
                        lhsT=OT[f][:, i * P:(i + 1) * P],
                        rhs=wo_t[f][:, n * NH:(n + 1) * NH],
                        start=(f == 0), stop=(f == FT - 1),
                    )
            # the end barrier waits for the LAST out-store to complete at
            # slow end-window DMA rate (~75GB/s): the final two i-tiles
            # store in quarter-chunks, with the last tile on the idle
            # gpsimd queue so the two trailing transfers run in parallel
            res = res_pool.tile([P, DIM], F32, tag="res", name=f"res{i}")
            nchunk = 4 if i >= IT - 2 else 2
            q = nc.gpsimd if i == IT - 1 else nc.sync
            CW = DIM // nchunk
            for hf in range(nchunk):
                sl = slice(hf * CW, (hf + 1) * CW)
                nc.vector.tensor_add(res[:, sl], ps[:, sl], bias_t[:, sl])
                q.dma_start(out=out_d[i * P:(i + 1) * P, sl],
                            in_=res[:, sl])

    nc.compile()
    return nc


class TileBuild:
    """TileContext + ExitStack pools in one with-block."""

    def __init__(self, nc):
        self.nc = nc
        self.ctx = ExitStack()
        self._tc_cm = tile.TileContext(nc)

    def __enter__(self):
        self.tc = self._tc_cm.__enter__()
        self.ctx.__enter__()
        return self

    def __exit__(self, *exc):
        self.ctx.__exit__(*exc)
        return self._tc_cm.__exit__(*exc)


_NC_CACHE = None


def _get_nc():
    global _NC_CACHE
    if _NC_CACHE is None:
        _NC_CACHE = build_attention_nc()
    return _NC_CACHE


def prepare_inputs(x, y, w_qkv, w_out, b_out):
    bf16 = ml_dtypes.bfloat16
    xT32 = np.ascontiguousarray(np.transpose(x, (0, 2, 1))).astype(np.float32)
    yT32 = np.ascontiguousarray(np.transpose(y, (0, 2, 1))).astype(np.float32)
    xT = xT32.astype(bf16)
    f8 = ml_dtypes.float8_e4m3
    wqkvT = np.ascontiguousarray(np.array(w_qkv, dtype=np.float32).T)
    # fp8 DoubleRow packing for the Q/K projection weights, scaled by WS
    # (softmax scale and 1/WS^2 are folded into the exp activation):
    # wqk8[t, a, p, plane, m] = WS * wqkvT[256a + 128*plane + p, col_t[m]]
    w8 = np.empty((DIM // P, DIM // 256, P, 2, 256), dtype=np.float32)
    for t in range(DIM // P):
        blk = np.concatenate(
            [wqkvT[:, t * P:(t + 1) * P],
             wqkvT[:, DIM + t * P:DIM + (t + 1) * P]], axis=1)  # [dim, 256]
        w8[t] = (WS * blk).reshape(DIM // 256, 2, P, 256).transpose(0, 2, 1, 3)
    # per-partition contiguous: row (t, p) = [a, plane, m]
    wqk8 = np.ascontiguousarray(
        w8.transpose(0, 2, 1, 3, 4).reshape((DIM // P) * P, 4 * 512)).astype(f8)
    wvT = np.ascontiguousarray(
        wqkvT[:, 2 * DIM:3 * DIM].reshape(DIM // P, P, DIM)
        .transpose(1, 0, 2).reshape(P, (DIM // P) * DIM)).astype(bf16)
    woutT = np.ascontiguousarray(np.array(w_out, dtype=np.float32).T).astype(bf16)
    biasb = np.ascontiguousarray(
        np.broadcast_to(np.array(b_out, dtype=np.float32), (P, DIM)))
    in_maps = []
    for i in range(x.shape[0]):
        in_maps.append({
            "xT": np.ascontiguousarray(xT[i]),
            "x8": np.ascontiguousarray(
                xT32[i].reshape(DIM // 256, 2, P, N)
                .transpose(2, 0, 1, 3).reshape(P, (DIM // 256) * 2 * N)
                ).astype(f8),
            "y8": np.ascontiguousarray(
                yT32[i].reshape(DIM // 256, 2, P, N)
                .transpose(2, 0, 1, 3).reshape(P, (DIM // 256) * 2 * N)
                ).astype(f8),
            "wqk8": wqk8,
            "wvT": wvT,
            "woutT": woutT,
            "biasb": biasb,
            "zpad": np.zeros((DH, N), dtype=bf16),
        })
    return in_maps


def kernel(x, y, w_qkv, w_out, b_out, trace=False):
    nc = _get_nc()
    in_maps = prepare_inputs(x, y, w_qkv, w_out, b_out)
    r = run_bass_kernel_spmd(nc, in_maps, core_ids=list(range(len(in_maps))),
                             trace=trace)
    out = np.stack([r.results[i]["out"] for i in range(len(in_maps))])
    if trace:
        kernel.last_results = r
    return out.astype(np.float32)

